# revision 22
# baseline (speedup 1.0000x reference)
"""Trainium2 Bass kernel for the autoregressive GRU decoder.

Reference computation (eval-mode Decoder):
  x0 = x[:, 30, :]                # only element of x ever used
  h0 = h[0]
  for t in 0..29:
      h = GRUCell(x_t, h)         # PyTorch gate layout [r, z, n]
      y_t = h @ W_out.T + b_out
      x_{t+1} = y_t               # linear feedback -> fold into weights
  out = stack(y_t)                # [B, 30, 32]

Because the feedback x_{t+1} = W_out @ h_t + b_out is linear, for t >= 1:
  gi_t = W_ih @ x_t + b_ih = (W_ih @ W_out) @ h_{t-1} + (W_ih @ b_out + b_ih)
so every step t >= 1 is a pure H->H recurrence; weights are folded on the
host.

Kernel structure (per core, pure data parallel over batch, 8 x 2048):
  - 30 steps fully unrolled, batch split in 4 chunks of 512 that pipeline
    against each other across the engines (PE/ACT/DVE/Pool all have
    per-step work; chunking overlaps them instead of phase-serializing,
    and the recurrence chain h' -> gate mm -> sigmoid_r -> u -> accum_v
    -> tanh -> w -> w2 -> h' gets 4 chunk-periods of slack).
  - state h^T kept in fp16 (DVE 2x perf modes); PSUM + biases f32.
  - PSUM: four [128, 512] f32 region tiles (r | z | d | c), each 1 bank,
    double-buffered = 8 banks exactly. Separate tiles per region because
    the tile framework's WAR tracking is tile-granular.
  - v = c + u is accumulated by the PE with an identity matmul
    (start=False) instead of a DVE op; tanh/sigmoid biases ride the ACT
    bias operand; u = (d + b_hn) * r is one DVE scalar_tensor_tensor.
  - w = h - n and w2 = z * w run on the otherwise-idle GpSimd engine.
  - y_t = W_out @ h_t is computed one step late (filling PE bubbles),
    squatting the drained sigma_z PSUM tile (off the recurrence chain),
    copied PSUM->SBUF as fp16 (DVE), DMA'd per chunk; b_out is added on
    the host, which also upcasts to f32.
"""

import numpy as np

B, T, I, H, SEQLEN = 16384, 60, 32, 128, 30
STEPS = T - SEQLEN  # 30
NCORES = 8
BC = B // NCORES  # 2048 batch rows per core
CH = BC // 4  # chunk = 512
MMN = 512  # matmul moving-operand free-dim limit (one PSUM bank of fp32)

LAST_RESULT = None  # BassKernelResults of the most recent run (for test.py)

_CACHE = {}

# number of bias columns in the f32 bias tile
NBIAS = 7  # b_r, b_z, b_hn, b_in, b0_r, b0_z, b0_in


def _build(repeats=1):
    from contextlib import ExitStack

    import concourse.bacc as bacc
    import concourse.bass as bass  # noqa: F401
    import concourse.mybir as mybir
    import concourse.tile as tile

    f32 = mybir.dt.float32
    f16 = mybir.dt.float16
    Alu = mybir.AluOpType
    Act = mybir.ActivationFunctionType

    nc = bacc.Bacc()

    # packed inputs:
    # cst  f16 [H, 4H | 2H | I | H]   = WA | WA0 | WoutT | I128
    # xw   f16 [I, BC | 3H]           = x0^T | W0
    # h0t  f16 [H, BC]
    # bias f32 [H, NBIAS]
    CW = 4 * H + 2 * H + I + H
    dcst = nc.dram_tensor("cst", [H, CW], f16, kind="ExternalInput")
    dxw = nc.dram_tensor("xw", [I, BC + 3 * H], f16, kind="ExternalInput")
    dh = nc.dram_tensor("h0t", [H, BC], f16, kind="ExternalInput")
    dbias = nc.dram_tensor("bias", [H, NBIAS], f32, kind="ExternalInput")
    dout = nc.dram_tensor("out", [STEPS, I * BC], f16, kind="ExternalOutput")

    with ExitStack() as ctx:
        tc = ctx.enter_context(tile.TileContext(nc))
        const = ctx.enter_context(tc.tile_pool(name="const", bufs=1))
        work = ctx.enter_context(tc.tile_pool(name="work", bufs=1))
        psum = ctx.enter_context(tc.tile_pool(name="psum", bufs=1, space="PSUM"))

        def load_const(dram, shape, name):
            t = const.tile(shape, dram.dtype, tag=name)
            nc.sync.dma_start(out=t[:], in_=dram[:, :])
            return t

        scst = load_const(dcst, [H, CW], "cst")
        sxw = load_const(dxw, [I, BC + 3 * H], "xw")
        h0 = load_const(dh, [H, BC], "h")
        sbias = load_const(dbias, [H, NBIAS], "bias")

        b_r = sbias[:, 0:1]
        b_z = sbias[:, 1:2]
        b_hn = sbias[:, 2:3]
        b_in = sbias[:, 3:4]
        b0_r = sbias[:, 4:5]
        b0_z = sbias[:, 5:6]
        b0_in = sbias[:, 6:7]

        A_r = scst[:, 0 * H : 1 * H]
        A_z = scst[:, 1 * H : 2 * H]
        A_hn = scst[:, 2 * H : 3 * H]
        A_in = scst[:, 3 * H : 4 * H]
        A0_r = scst[:, 4 * H : 5 * H]
        A0_z = scst[:, 5 * H : 6 * H]
        WoutT = scst[:, 6 * H : 6 * H + I]
        I128 = scst[:, 6 * H + I : 6 * H + I + H]
        sx0 = sxw[:, 0:BC]
        W0_r = sxw[:, BC + 0 * H : BC + 1 * H]
        W0_z = sxw[:, BC + 1 * H : BC + 2 * H]
        W0_n = sxw[:, BC + 2 * H : BC + 3 * H]

        def run_once():
            # 4 chunks of 512 per step. Each PSUM region (r | z | d | c)
            # is a [128, 512] f32 tile = 1 bank, double-buffered (8 banks
            # total), so consecutive chunks never collide in PSUM and the
            # only serialization left is the h-recurrence chain itself,
            # which spans 4 chunk-periods of slack. Separate tiles per
            # region/chunk because WAR tracking is tile-granular.
            def chunk_ops(t, q, first, h_prev, h_cur):
                Pr = psum.tile([128, CH], f32, tag="Pr", bufs=2, name="Pr")
                Pz = psum.tile([128, CH], f32, tag="Pz", bufs=2, name="Pz")
                Pd = psum.tile([128, CH], f32, tag="Pd", bufs=2, name="Pd")
                Pc = psum.tile([128, CH], f32, tag="Pc", bufs=2, name="Pc")
                xl = slice(q * CH, (q + 1) * CH)

                # PE: gate matmuls (FD=512 -> single matmul per gate)
                if first:
                    mm(Pr[:, :], A0_r, h_prev[:, :], start=True, stop=False)
                    mm(Pr[:, :], W0_r, sx0[:, xl], start=False, stop=True)
                    mm(Pz[:, :], A0_z, h_prev[:, :], start=True, stop=False)
                    mm(Pz[:, :], W0_z, sx0[:, xl], start=False, stop=True)
                    mm(Pd[:, :], A_hn, h_prev[:, :])
                    mm(Pc[:, :], W0_n, sx0[:, xl], start=True, stop=False)
                else:
                    mm(Pr[:, :], A_r, h_prev[:, :])
                    mm(Pz[:, :], A_z, h_prev[:, :])
                    mm(Pd[:, :], A_hn, h_prev[:, :])
                    mm(Pc[:, :], A_in, h_prev[:, :], start=True, stop=False)

                r_sb = work.tile([128, CH], f16, tag="r", bufs=2, name="r")
                nc.scalar.activation(
                    r_sb[:], Pr[:, :], Act.Sigmoid, bias=b0_r if first else b_r
                )
                z_sb = work.tile([128, CH], f16, tag="z", bufs=2, name="z")
                nc.scalar.activation(
                    z_sb[:], Pz[:, :], Act.Sigmoid, bias=b0_z if first else b_z
                )

                if not first:
                    # y of the PREVIOUS step squats Pz (partitions 0..I)
                    # after sigma_z drains it; Pz is off the h-recurrence.
                    mm(Pz[0:I, :], WoutT, h_prev[:, :])

                u_sb = work.tile([128, CH], f16, tag="u", bufs=2, name="u")
                nc.vector.scalar_tensor_tensor(
                    u_sb[:], Pd[:, :], b_hn, r_sb[:], Alu.add, Alu.mult
                )
                # PE: v = c + u via identity matmul (closes the c group)
                mm(Pc[:, :], I128, u_sb[:, :], start=False, stop=True)

                n_sb = work.tile([128, CH], f16, tag="n", bufs=2, name="n")
                nc.scalar.activation(
                    n_sb[:], Pc[:, :], Act.Tanh, bias=b0_in if first else b_in
                )

                if not first:
                    y_out(t - 1, q, Pz)

                # h' = n + z*(h-n); z-path ops on the otherwise-idle
                # GpSimd engine, the final add on DVE
                w_sb = work.tile([128, CH], f16, tag="w", bufs=2, name="w")
                nc.gpsimd.tensor_tensor(w_sb[:], h_prev[:, :], n_sb[:], Alu.subtract)
                nc.gpsimd.tensor_tensor(w_sb[:], z_sb[:], w_sb[:], Alu.mult)
                nc.vector.tensor_tensor(h_cur[:, :], n_sb[:], w_sb[:], Alu.add)

            def y_out(t_out, q, Pz):
                """DVE copy PSUM->SBUF f16 + DMA chunk q of step t_out."""
                y_sb = work.tile([I, CH], f16, tag="y", bufs=2, name="y")
                nc.vector.tensor_copy(y_sb[:], Pz[0:I, :])
                dst = dout[t_out : t_out + 1, :].rearrange(
                    "o (p f) -> p o f", p=I
                )[:, :, q * CH : (q + 1) * CH]
                nc.sync.dma_start(out=dst, in_=y_sb[:, None, :])

            def mm(dst, A_, rhs, start=True, stop=True):
                nc.tensor.matmul(dst, A_, rhs, start=start, stop=stop)

            NCH = BC // CH
            h_tiles = [[h0[:, q * CH : (q + 1) * CH] for q in range(NCH)]]
            for k in range(NCH * STEPS):
                t, q = divmod(k, NCH)
                first = t == 0
                if q == 0:
                    h_tiles.append([
                        work.tile([H, CH], f16, tag=f"hst{i}", bufs=3,
                                  name=f"hst{i}")
                        for i in range(NCH)
                    ])
                chunk_ops(t, q, first, h_tiles[t][q], h_tiles[t + 1][q])

            # epilogue: y for the last step
            for q in range(NCH):
                Pz = psum.tile([128, CH], f32, tag="Pz", bufs=2, name="Pz")
                mm(Pz[0:I, :], WoutT, h_tiles[STEPS][q][:, :])
                y_out(STEPS - 1, q, Pz)

        if repeats == 1:
            run_once()
        else:
            # benchmarking only: outer HARDWARE loop keeps the instruction
            # stream fixed while repeating the full computation, so
            # (wall(R) - wall(1)) / (R-1) beats the axon-tunnel noise.
            with tc.For_i(0, repeats, 1):
                nc.sync.dma_start(out=h0[:], in_=dh[:, :])
                run_once()

    return nc


def _host_prep(x, h, W_ih, W_hh, b_ih, b_hh, W_out, b_out):
    """Fold weights on the host (float64 for exactness), build per-core maps."""
    x = np.asarray(x, dtype=np.float32)
    h = np.asarray(h, dtype=np.float32)
    W_ih = np.asarray(W_ih, dtype=np.float64)
    W_hh = np.asarray(W_hh, dtype=np.float64)
    b_ih = np.asarray(b_ih, dtype=np.float64)
    b_hh = np.asarray(b_hh, dtype=np.float64)
    W_out = np.asarray(W_out, dtype=np.float64)
    b_out = np.asarray(b_out, dtype=np.float64)

    W_ih_eff = W_ih @ W_out  # [3H, H]
    b_ih_eff = W_ih @ b_out + b_ih  # [3H]

    def cvt16(a):
        return np.ascontiguousarray(a, dtype=np.float16)

    WA = cvt16(
        np.concatenate(
            [
                (W_hh[0:H] + W_ih_eff[0:H]).T,
                (W_hh[H : 2 * H] + W_ih_eff[H : 2 * H]).T,
                W_hh[2 * H : 3 * H].T,
                W_ih_eff[2 * H : 3 * H].T,
            ],
            axis=1,
        )
    )  # [H, 4H]
    WA0 = cvt16(np.concatenate([W_hh[0:H].T, W_hh[H : 2 * H].T], axis=1))
    W0 = cvt16(
        np.concatenate([W_ih[0:H].T, W_ih[H : 2 * H].T, W_ih[2 * H : 3 * H].T], axis=1)
    )  # [I, 3H]
    WoutT = cvt16(W_out.T)  # [H, I]
    I128 = cvt16(np.eye(H))

    BIAS = np.ascontiguousarray(
        np.stack(
            [
                b_hh[0:H] + b_ih_eff[0:H],  # b_r
                b_hh[H : 2 * H] + b_ih_eff[H : 2 * H],  # b_z
                b_hh[2 * H : 3 * H],  # b_hn
                b_ih_eff[2 * H : 3 * H],  # b_in
                b_hh[0:H] + b_ih[0:H],  # b0_r
                b_hh[H : 2 * H] + b_ih[H : 2 * H],  # b0_z
                b_ih[2 * H : 3 * H],  # b0_in
            ],
            axis=1,
        ),
        dtype=np.float32,
    )  # [H, NBIAS]

    x0T = cvt16(x[:, SEQLEN, :].T)  # [I, B]
    h0T = cvt16(h[0].T)  # [H, B]

    CST = np.concatenate([WA, WA0, WoutT, I128], axis=1)  # [H, CW] f16
    in_maps = []
    for core in range(NCORES):
        cs = slice(core * BC, (core + 1) * BC)
        in_maps.append(
            {
                "cst": CST,
                "xw": np.concatenate(
                    [np.ascontiguousarray(x0T[:, cs]), W0], axis=1
                ),
                "h0t": np.ascontiguousarray(h0T[:, cs]),
                "bias": BIAS,
            }
        )
    return in_maps


def _unshuffle(out_dev):
    """[STEPS, I*BC] device layout (f16) -> [BC, STEPS, I] f16."""
    x = out_dev.reshape(STEPS, I, BC)
    return np.ascontiguousarray(x.transpose(2, 0, 1))


def _postprocess(raw, inputs):
    """f16 device output -> f32 + b_out."""
    b_out = np.asarray(inputs["b_out"], dtype=np.float32)
    return raw.astype(np.float32) + b_out


def _get_nc(repeats=1):
    key = repeats
    if key not in _CACHE:
        nc = _build(repeats)
        # Bacc needs explicit finalize (wait-splitting, reg alloc);
        # run_bass_via_pjrt serializes the module as-is.
        nc.finalize()
        _CACHE[key] = nc
    return _CACHE[key]


def run(in_maps, repeats=1):
    global LAST_RESULT
    from concourse.bass_utils import run_bass_kernel_spmd

    nc = _get_nc(repeats)
    res = run_bass_kernel_spmd(nc, in_maps, core_ids=list(range(NCORES)))
    LAST_RESULT = res
    return res


def gather(res):
    return np.concatenate([_unshuffle(r["out"]) for r in res.results], axis=0)


def kernel(x, h, W_ih, W_hh, b_ih, b_hh, W_out, b_out):
    in_maps = _host_prep(x, h, W_ih, W_hh, b_ih, b_hh, W_out, b_out)
    res = run(in_maps, repeats=1)
    raw = gather(res)
    return _postprocess(raw, {"b_out": b_out})


# revision 25
# speedup vs baseline: 1.0262x; 1.0262x over previous
"""Trainium2 Bass kernel for the autoregressive GRU decoder.

Reference computation (eval-mode Decoder):
  x0 = x[:, 30, :]                # only element of x ever used
  h0 = h[0]
  for t in 0..29:
      h = GRUCell(x_t, h)         # PyTorch gate layout [r, z, n]
      y_t = h @ W_out.T + b_out
      x_{t+1} = y_t               # linear feedback -> fold into weights
  out = stack(y_t)                # [B, 30, 32]

Because the feedback x_{t+1} = W_out @ h_t + b_out is linear, for t >= 1:
  gi_t = W_ih @ x_t + b_ih = (W_ih @ W_out) @ h_{t-1} + (W_ih @ b_out + b_ih)
so every step t >= 1 is a pure H->H recurrence; weights are folded on the
host.

Kernel structure (per core, pure data parallel over batch, 8 x 2048):
  - 30 steps fully unrolled, batch split in 4 chunks of 512 that pipeline
    against each other across the engines (PE/ACT/DVE/Pool all have
    per-step work; chunking overlaps them instead of phase-serializing,
    and the recurrence chain h' -> gate mm -> sigmoid_r -> u -> accum_v
    -> tanh -> w -> w2 -> h' gets 4 chunk-periods of slack).
  - state h^T kept in fp16 (DVE 2x perf modes); PSUM + biases f32.
  - PSUM: four [128, 512] f32 region tiles (r | z | d | c), each 1 bank,
    double-buffered = 8 banks exactly. Separate tiles per region because
    the tile framework's WAR tracking is tile-granular.
  - v = c + u is accumulated by the PE with an identity matmul
    (start=False) instead of a DVE op; tanh/sigmoid biases ride the ACT
    bias operand; u = (d + b_hn) * r is one DVE scalar_tensor_tensor.
  - w = h - n and w2 = z * w run on the otherwise-idle GpSimd engine.
  - y_t = W_out @ h_t is computed one step late (filling PE bubbles),
    squatting the drained sigma_z PSUM tile (off the recurrence chain),
    copied PSUM->SBUF as fp16 (DVE), DMA'd per chunk; b_out is added on
    the host, which also upcasts to f32.
"""

import numpy as np

B, T, I, H, SEQLEN = 16384, 60, 32, 128, 30
STEPS = T - SEQLEN  # 30
NCORES = 8
BC = B // NCORES  # 2048 batch rows per core
CH = BC // 4  # chunk = 512
MMN = 512  # matmul moving-operand free-dim limit (one PSUM bank of fp32)

LAST_RESULT = None  # BassKernelResults of the most recent run (for test.py)

_CACHE = {}

# number of bias columns in the f32 bias tile
NBIAS = 7  # b_r, b_z, b_hn, b_in, b0_r, b0_z, b0_in


def _build(repeats=1):
    from contextlib import ExitStack

    import concourse.bacc as bacc
    import concourse.bass as bass  # noqa: F401
    import concourse.mybir as mybir
    import concourse.tile as tile

    f32 = mybir.dt.float32
    f16 = mybir.dt.float16
    Alu = mybir.AluOpType
    Act = mybir.ActivationFunctionType

    nc = bacc.Bacc()

    # packed inputs:
    # cst  f16 [H, 4H | 2H | I | H]   = WA | WA0 | WoutT | I128
    # xw   f16 [I, BC | 3H]           = x0^T | W0
    # h0t  f16 [H, BC]
    # bias f32 [H, NBIAS]
    CW = 4 * H + 2 * H + I + H
    dcst = nc.dram_tensor("cst", [H, CW], f16, kind="ExternalInput")
    dxw = nc.dram_tensor("xw", [I, BC + 3 * H], f16, kind="ExternalInput")
    dh = nc.dram_tensor("h0t", [H, BC], f16, kind="ExternalInput")
    dbias = nc.dram_tensor("bias", [H, NBIAS], f32, kind="ExternalInput")
    dout = nc.dram_tensor("out", [STEPS, I * BC], f16, kind="ExternalOutput")

    with ExitStack() as ctx:
        tc = ctx.enter_context(tile.TileContext(nc))
        const = ctx.enter_context(tc.tile_pool(name="const", bufs=1))
        work = ctx.enter_context(tc.tile_pool(name="work", bufs=1))
        psum = ctx.enter_context(tc.tile_pool(name="psum", bufs=1, space="PSUM"))

        def load_const(dram, shape, name):
            t = const.tile(shape, dram.dtype, tag=name)
            nc.sync.dma_start(out=t[:], in_=dram[:, :])
            return t

        scst = load_const(dcst, [H, CW], "cst")
        sxw = load_const(dxw, [I, BC + 3 * H], "xw")
        h0 = load_const(dh, [H, BC], "h")
        sbias = load_const(dbias, [H, NBIAS], "bias")

        b_r = sbias[:, 0:1]
        b_z = sbias[:, 1:2]
        b_hn = sbias[:, 2:3]
        b_in = sbias[:, 3:4]
        b0_r = sbias[:, 4:5]
        b0_z = sbias[:, 5:6]
        b0_in = sbias[:, 6:7]

        A_r = scst[:, 0 * H : 1 * H]
        A_z = scst[:, 1 * H : 2 * H]
        A_hn = scst[:, 2 * H : 3 * H]
        A_in = scst[:, 3 * H : 4 * H]
        A0_r = scst[:, 4 * H : 5 * H]
        A0_z = scst[:, 5 * H : 6 * H]
        WoutT = scst[:, 6 * H : 6 * H + I]
        I128 = scst[:, 6 * H + I : 6 * H + I + H]
        sx0 = sxw[:, 0:BC]
        W0_r = sxw[:, BC + 0 * H : BC + 1 * H]
        W0_z = sxw[:, BC + 1 * H : BC + 2 * H]
        W0_n = sxw[:, BC + 2 * H : BC + 3 * H]

        def run_once():
            # 4 chunks of 512 per step. Each PSUM region (r | z | d | c)
            # is a [128, 512] f32 tile = 1 bank, double-buffered (8 banks
            # total), so consecutive chunks never collide in PSUM and the
            # only serialization left is the h-recurrence chain itself,
            # which spans 4 chunk-periods of slack. Separate tiles per
            # region/chunk because WAR tracking is tile-granular.
            def chunk_ops(t, q, first, h_prev, h_cur):
                Pr = psum.tile([128, CH], f32, tag="Pr", bufs=2, name="Pr")
                Pz = psum.tile([128, CH], f32, tag="Pz", bufs=2, name="Pz")
                Pd = psum.tile([128, CH], f32, tag="Pd", bufs=2, name="Pd")
                Pc = psum.tile([128, CH], f32, tag="Pc", bufs=2, name="Pc")
                xl = slice(q * CH, (q + 1) * CH)

                # PE: gate matmuls (FD=512 -> single matmul per gate)
                if first:
                    mm(Pr[:, :], A0_r, h_prev[:, :], start=True, stop=False)
                    mm(Pr[:, :], W0_r, sx0[:, xl], start=False, stop=True)
                    mm(Pz[:, :], A0_z, h_prev[:, :], start=True, stop=False)
                    mm(Pz[:, :], W0_z, sx0[:, xl], start=False, stop=True)
                    mm(Pd[:, :], A_hn, h_prev[:, :])
                    mm(Pc[:, :], W0_n, sx0[:, xl], start=True, stop=False)
                else:
                    mm(Pr[:, :], A_r, h_prev[:, :])
                    mm(Pz[:, :], A_z, h_prev[:, :])
                    mm(Pd[:, :], A_hn, h_prev[:, :])
                    mm(Pc[:, :], A_in, h_prev[:, :], start=True, stop=False)

                r_sb = work.tile([128, CH], f16, tag="r", bufs=2, name="r")
                nc.scalar.activation(
                    r_sb[:], Pr[:, :], Act.Sigmoid, bias=b0_r if first else b_r
                )
                z_sb = work.tile([128, CH], f16, tag="z", bufs=2, name="z")
                nc.scalar.activation(
                    z_sb[:], Pz[:, :], Act.Sigmoid, bias=b0_z if first else b_z
                )

                if not first:
                    # y of the PREVIOUS step squats Pz (partitions 0..I)
                    # after sigma_z drains it; Pz is off the h-recurrence.
                    mm(Pz[0:I, :], WoutT, h_prev[:, :])

                u_sb = work.tile([128, CH], f16, tag="u", bufs=2, name="u")
                nc.vector.scalar_tensor_tensor(
                    u_sb[:], Pd[:, :], b_hn, r_sb[:], Alu.add, Alu.mult
                )
                # PE: v = c + u via identity matmul (closes the c group)
                mm(Pc[:, :], I128, u_sb[:, :], start=False, stop=True)

                n_sb = work.tile([128, CH], f16, tag="n", bufs=2, name="n")
                nc.scalar.activation(
                    n_sb[:], Pc[:, :], Act.Tanh, bias=b0_in if first else b_in
                )

                if not first:
                    y_out(t - 1, q, Pz)

                # h' = n + z*(h-n); z-path ops on the otherwise-idle
                # GpSimd engine, the final add on DVE
                w_sb = work.tile([128, CH], f16, tag="w", bufs=2, name="w")
                nc.gpsimd.tensor_tensor(w_sb[:], h_prev[:, :], n_sb[:], Alu.subtract)
                nc.gpsimd.tensor_tensor(w_sb[:], z_sb[:], w_sb[:], Alu.mult)
                nc.vector.tensor_tensor(h_cur[:, :], n_sb[:], w_sb[:], Alu.add)

            def y_out(t_out, q, Pz):
                """DVE copy PSUM->SBUF f16 + DMA chunk q of step t_out."""
                y_sb = work.tile([I, CH], f16, tag="y", bufs=2, name="y")
                nc.vector.tensor_copy(y_sb[:], Pz[0:I, :])
                dst = dout[t_out : t_out + 1, :].rearrange(
                    "o (p f) -> p o f", p=I
                )[:, :, q * CH : (q + 1) * CH]
                nc.sync.dma_start(out=dst, in_=y_sb[:, None, :])

            def mm(dst, A_, rhs, start=True, stop=True):
                nc.tensor.matmul(dst, A_, rhs, start=start, stop=stop)

            NCH = BC // CH
            h_tiles = [[h0[:, q * CH : (q + 1) * CH] for q in range(NCH)]]
            for k in range(NCH * STEPS):
                t, q = divmod(k, NCH)
                first = t == 0
                if q == 0:
                    h_tiles.append([
                        work.tile([H, CH], f16, tag=f"hst{i}", bufs=3,
                                  name=f"hst{i}")
                        for i in range(NCH)
                    ])
                chunk_ops(t, q, first, h_tiles[t][q], h_tiles[t + 1][q])

            # epilogue: y for the last step
            for q in range(NCH):
                Pz = psum.tile([128, CH], f32, tag="Pz", bufs=2, name="Pz")
                mm(Pz[0:I, :], WoutT, h_tiles[STEPS][q][:, :])
                y_out(STEPS - 1, q, Pz)

        if repeats == 1:
            run_once()
        else:
            # benchmarking only: outer HARDWARE loop keeps the instruction
            # stream fixed while repeating the full computation, so
            # (wall(R) - wall(1)) / (R-1) beats the axon-tunnel noise.
            with tc.For_i(0, repeats, 1):
                nc.sync.dma_start(out=h0[:], in_=dh[:, :])
                run_once()

    return nc


def _host_prep(x, h, W_ih, W_hh, b_ih, b_hh, W_out, b_out):
    """Fold weights on the host (float64 for exactness), build per-core maps."""
    x = np.asarray(x, dtype=np.float32)
    h = np.asarray(h, dtype=np.float32)
    W_ih = np.asarray(W_ih, dtype=np.float64)
    W_hh = np.asarray(W_hh, dtype=np.float64)
    b_ih = np.asarray(b_ih, dtype=np.float64)
    b_hh = np.asarray(b_hh, dtype=np.float64)
    W_out = np.asarray(W_out, dtype=np.float64)
    b_out = np.asarray(b_out, dtype=np.float64)

    W_ih_eff = W_ih @ W_out  # [3H, H]
    b_ih_eff = W_ih @ b_out + b_ih  # [3H]

    def cvt16(a):
        return np.ascontiguousarray(a, dtype=np.float16)

    WA = cvt16(
        np.concatenate(
            [
                (W_hh[0:H] + W_ih_eff[0:H]).T,
                (W_hh[H : 2 * H] + W_ih_eff[H : 2 * H]).T,
                W_hh[2 * H : 3 * H].T,
                W_ih_eff[2 * H : 3 * H].T,
            ],
            axis=1,
        )
    )  # [H, 4H]
    WA0 = cvt16(np.concatenate([W_hh[0:H].T, W_hh[H : 2 * H].T], axis=1))
    W0 = cvt16(
        np.concatenate([W_ih[0:H].T, W_ih[H : 2 * H].T, W_ih[2 * H : 3 * H].T], axis=1)
    )  # [I, 3H]
    WoutT = cvt16(W_out.T)  # [H, I]
    I128 = cvt16(np.eye(H))

    BIAS = np.ascontiguousarray(
        np.stack(
            [
                b_hh[0:H] + b_ih_eff[0:H],  # b_r
                b_hh[H : 2 * H] + b_ih_eff[H : 2 * H],  # b_z
                b_hh[2 * H : 3 * H],  # b_hn
                b_ih_eff[2 * H : 3 * H],  # b_in
                b_hh[0:H] + b_ih[0:H],  # b0_r
                b_hh[H : 2 * H] + b_ih[H : 2 * H],  # b0_z
                b_ih[2 * H : 3 * H],  # b0_in
            ],
            axis=1,
        ),
        dtype=np.float32,
    )  # [H, NBIAS]

    x0T = cvt16(x[:, SEQLEN, :].T)  # [I, B]
    h0T = cvt16(h[0].T)  # [H, B]

    CST = np.concatenate([WA, WA0, WoutT, I128], axis=1)  # [H, CW] f16
    in_maps = []
    for core in range(NCORES):
        cs = slice(core * BC, (core + 1) * BC)
        in_maps.append(
            {
                "cst": CST,
                "xw": np.concatenate(
                    [np.ascontiguousarray(x0T[:, cs]), W0], axis=1
                ),
                "h0t": np.ascontiguousarray(h0T[:, cs]),
                "bias": BIAS,
            }
        )
    return in_maps


def _unshuffle(out_dev):
    """[STEPS, I*BC] device layout (f16) -> [BC, STEPS, I] f16."""
    x = out_dev.reshape(STEPS, I, BC)
    return np.ascontiguousarray(x.transpose(2, 0, 1))


def _postprocess(raw, inputs):
    """f16 device output -> f32 + b_out."""
    b_out = np.asarray(inputs["b_out"], dtype=np.float32)
    return raw.astype(np.float32) + b_out


def _get_nc(repeats=1):
    key = repeats
    if key not in _CACHE:
        nc = _build(repeats)
        # Bacc needs explicit finalize (wait-splitting, reg alloc);
        # run_bass_via_pjrt serializes the module as-is.
        nc.finalize()
        _CACHE[key] = nc
    return _CACHE[key]


def run(in_maps, repeats=1):
    global LAST_RESULT
    from concourse.bass_utils import run_bass_kernel_spmd

    nc = _get_nc(repeats)
    res = run_bass_kernel_spmd(nc, in_maps, core_ids=list(range(NCORES)))
    LAST_RESULT = res
    return res


def gather(res):
    return np.concatenate([_unshuffle(r["out"]) for r in res.results], axis=0)


def kernel(x, h, W_ih, W_hh, b_ih, b_hh, W_out, b_out):
    in_maps = _host_prep(x, h, W_ih, W_hh, b_ih, b_hh, W_out, b_out)
    res = run(in_maps, repeats=1)
    raw = gather(res)
    return _postprocess(raw, {"b_out": b_out})
